# revision 29
# baseline (speedup 1.0000x reference)
"""Trainium2 kernel for nn_AttentionFusion (dense_transformer).

Math: the reference MHA has seq_len 1 for q and kv, so softmax over the
single kv position is identically 1.0 and the attention output equals the
value projection. The whole module therefore collapses (exactly, up to fp
rounding) to one affine map per input stream:

    out = relu(audio @ Waa.T + visual @ Wva.T + b)

with
    Wvo = Wo @ Wi[2E:]             bvo = Wo @ bi[2E:] + bo
    Wfv = Wf[:, :E] @ Wvo          Wfa = Wf[:, E:] @ Wvo
    Waa = Wfa @ Wa                 Wva = Wfv @ Wv
    b   = Wfa @ ba + Wfv @ bv + (Wf[:, :E] + Wf[:, E:]) @ bvo + bf

Weight composition is done on host in float64 (cheap: ~15 GFLOP), the big
GEMM (32768 x 4096 @ 4096 x 1024, 275 GFLOP) runs on 8 NeuronCores, batch
sharded (pure data parallel per the sharding hint).

Precision/speed (HW-measured, warm clock 2.4 GHz):
  - fp32r matmul: 1.28 cyc/row on real HW (cost model claims 1.0) -> 272ns
    per 512-row matmul. bf16: true 1 cyc/row -> 216ns. rel err 2.0e-3.
  - fp8e4 (e4m3) DoubleRow: one instruction covers TWO 128-row k-chunks
    ([p, 2, f] operand layout) at the SAME per-instruction cost as one
    bf16 matmul -> 2x on the contraction dim. The composed weights
    (~2e-3 std) sit below e4m3's subnormal floor (2^-6), so operands are
    pre-scaled on host: w*32 and x/32 (exact powers of 2, product
    unchanged). The LAST KF8 = 2*FP8_PAIRS k-chunks run in fp8; rel err
    measured on the real inputs: 1.30e-2 (PAIRS=2) / 1.56e-2 (PAIRS=3)
    vs the 2e-2 gate.
  - P0 power throttling can drop the PE to ~2.0 GHz for a whole run
    (everything uniformly ~20% slower); ignore single-run regressions.

Device layout per core (bf16 path + fp8 tail):
    xt  [KB8=KO_BF*128, BC] bf16 - bf16 k-chunks, feature-major
    x8t [PAIRS, 128, 2, BC] fp8  - paired fp8 k-chunks (x/32)
    wt  [KB8, E]            bf16 - composed weight (replicated)
    w8t [PAIRS, 128, 2, E]  fp8  - paired fp8 weight chunks (w*32)
    bias[P, E]              f32  - row-replicated bias
    out [BC, E]             f32  - natural layout

Schedule: 512-row batch tiles, subtile (b) OUTER / contraction (k) INNER:
each 128-row subtile owns one [128, 1024] PSUM tile (2 banks) and its full
accumulation (26 bf16 + 2x3 fp8 DoubleRow matmuls) completes a
quarter-tile early, so the drain (DVE bias-add + DVE tensor_scalar_max
Relu -> one 512 KB DMA with 4 KB-contiguous rows) overlaps the next
subtile's compute and PSUM banks recycle without WAR stalls. The last
512-row tile is split into two 256-row half-tiles. 256-row tiles
everywhere were tried and regressed: PE consumes only ~864ns/k there vs
~1110ns/k of first-sweep DMA, starving the PE early on. Fixed ~8.6us of
NEFF init (DMA ring setup + const memsets) precedes the first data DMA
regardless of schedule.
"""

import os
import sys

import numpy as np

sys.path.insert(0, "/opt/trn_rl_repo")

import concourse.bacc as bacc
import concourse.mybir as mybir
import concourse.tile as tile
from concourse.bass_utils import run_bass_kernel_spmd

N_CORES = 8
B = 32768
BC = B // N_CORES  # 4096 batch rows per core
K = 4096           # 2048 audio + 2048 visual features
E = 1024
P = 128

KO = K // P        # 32 contraction tiles
NB = 512           # batch tile per iteration; also PSUM free width (= E/2)
NBT = BC // NB     # 8 batch tiles per core
B4 = NB // P       # 4 batch subtiles (PSUM partition dim)
M2 = E // NB       # 2 outfeat halves (PSUM free dim)

DT_NAME = os.environ.get("KMM_DTYPE", "bf16")
FP8_PAIRS = int(os.environ.get("KMM_FP8_PAIRS", "3"))  # 0 disables fp8 tail
FP8_SCALE = 32.0   # w*32, x/32: centers both operands in e4m3 normal range
KO_BF = KO - 2 * FP8_PAIRS  # leading bf16 k-chunks
KB8 = KO_BF * P

_NC_CACHE = {}
LAST_RESULTS = None  # stashed BassKernelResults for test.py introspection

# Note: walrus's --enable-ldw-opt=true was tested (dedupes the shared-lhsT
# LDWEIGHTS pairs) but measured SLOWER: the standalone-LW form loses the
# fused matmul's background weight-buffer pipelining (+12us PE).


def _build_nc(dt_name, fp8_pairs):
    mm_dt = {
        "f32": mybir.dt.float32,
        "f32r": mybir.dt.float32r,
        "bf16": mybir.dt.bfloat16,
    }[dt_name]
    f32 = mybir.dt.float32
    fp8 = mybir.dt.float8e4
    ko_bf = KO - 2 * fp8_pairs

    nc = bacc.Bacc("TRN2", debug=False, target_bir_lowering=False)
    xt = nc.dram_tensor("xt", [ko_bf * P, BC], mm_dt, kind="ExternalInput").ap()
    wt = nc.dram_tensor("wt", [ko_bf * P, E], mm_dt, kind="ExternalInput").ap()
    bias = nc.dram_tensor("bias", [P, E], f32, kind="ExternalInput").ap()
    out = nc.dram_tensor("out", [BC, E], f32, kind="ExternalOutput").ap()
    if fp8_pairs:
        x8t = nc.dram_tensor(
            "x8t", [fp8_pairs, P, 2, BC], fp8, kind="ExternalInput"
        ).ap()
        # Same [P, pairs, 2, E] layout as the SBUF tile so the single
        # whole-tensor DMA maps elements 1:1.
        w8t = nc.dram_tensor(
            "w8t", [P, fp8_pairs, 2, E], fp8, kind="ExternalInput"
        ).ap()

    with tile.TileContext(nc) as tc:
        with (
            tc.tile_pool(name="wpool", bufs=1) as wpool,
            tc.tile_pool(name="xpool", bufs=56) as xpool,
            tc.tile_pool(name="x8pool", bufs=6) as x8pool,
            tc.tile_pool(name="opool", bufs=6) as opool,
            tc.tile_pool(name="pspool", bufs=4, space="PSUM") as pspool,
        ):
            # The DMA path is one FIFO queue fanned over 16 engines (~300-346
            # GB/s measured; bigger transfers amortize per-DMA overhead).
            # Order the preamble just-in-time for batch tile 0's k-sweep.
            wt_sb = wpool.tile([P, ko_bf, E], mm_dt)
            wt_r = wt.rearrange("(ko ki) e -> ki ko e", ki=P)
            if fp8_pairs:
                w8_sb = wpool.tile([P, fp8_pairs, 2, E], fp8)
            xch0 = {}
            x80 = {}
            # First matmul needs xch(n0,k0) + wt[k0, 0:512] only: emit the
            # first wt chunk in halves so its DMA completes sooner (PE
            # starts ~4us instead of ~9us).
            for k in range(8):
                xch = xpool.tile([P, NB], mm_dt, tag="xch")
                nc.sync.dma_start(xch, xt[k * P : (k + 1) * P, 0:NB])
                xch0[k] = xch
                if k == 0:
                    nc.sync.dma_start(wt_sb[:, 0, 0:NB], wt_r[:, 0, 0:NB])
                    nc.sync.dma_start(wt_sb[:, 0, NB:E], wt_r[:, 0, NB:E])
                else:
                    nc.sync.dma_start(wt_sb[:, k], wt_r[:, k])

            bias_sb = wpool.tile([P, E], f32)
            for k in range(8, ko_bf):
                xch = xpool.tile([P, NB], mm_dt, tag="xch")
                nc.sync.dma_start(xch, xt[k * P : (k + 1) * P, 0:NB])
                xch0[k] = xch
                if k % 4 == 0:
                    # 1 MB weight chunks (4 ko's): fewer, larger transfers
                    # raise effective DMA bandwidth in the bandwidth-bound
                    # first k-sweep.
                    nc.sync.dma_start(
                        wt_sb[:, k : min(k + 4, ko_bf)],
                        wt_r[:, k : min(k + 4, ko_bf)],
                    )
                if k == 11:
                    # Bias early enough for the first tile's drains but off
                    # the first weight chunks' critical path.
                    nc.sync.dma_start(bias_sb, bias)
                if fp8_pairs and k == 13:
                    nc.sync.dma_start(w8_sb, w8t)
                if fp8_pairs and k == 15:
                    for q in range(fp8_pairs):
                        x8 = x8pool.tile([P, 2, NB], fp8, tag="x8")
                        nc.sync.dma_start(x8, x8t[q][:, :, 0:NB])
                        x80[q] = x8

            # 512-row batch tiles (8 PSUM banks each); the LAST tile is split
            # into two 256-row half-tiles (4 banks each) so the exposed final
            # drain chain after the last matmul halves.
            tiles = [(n * NB, B4) for n in range(NBT - 1)]
            tiles += [(BC - NB, B4 // 2), (BC - NB // 2, B4 // 2)]
            for n0, nb4 in tiles:
                # One [P, 1024] PSUM tile (2 banks) per batch subtile:
                # matmuls write 512-wide halves, but the drain is a single
                # [128, 1024] DVE add + Relu + one 512 KB DMA whose DRAM rows
                # are fully contiguous (4 KB lines).
                #
                # b OUTER / k INNER: each subtile's accumulation (26 bf16 +
                # 3 fp8 DoubleRow matmuls per outfeat half) finishes a
                # quarter-tile early, so its drain overlaps the next
                # subtile's compute and PSUM banks are free long before the
                # next tile reuses them (no boundary WAR stalls).
                xchs = {}
                for k in range(ko_bf):
                    if n0 == 0:
                        xchs[k] = xch0[k]
                    else:
                        xchs[k] = xpool.tile(
                            [P, NB], mm_dt, tag="xch", name=f"x_{n0}_{k}"
                        )
                        nc.sync.dma_start(
                            xchs[k][:, 0 : nb4 * P],
                            xt[k * P : (k + 1) * P, n0 : n0 + nb4 * P],
                        )
                x8s = {}
                for q in range(fp8_pairs):
                    if n0 == 0:
                        x8s[q] = x80[q]
                    else:
                        x8s[q] = x8pool.tile(
                            [P, 2, NB], fp8, tag="x8", name=f"x8_{n0}_{q}"
                        )
                        nc.sync.dma_start(
                            x8s[q][:, :, 0 : nb4 * P],
                            x8t[q][:, :, n0 : n0 + nb4 * P],
                        )
                for b in range(nb4):
                    ps = pspool.tile([P, E], f32, tag="ps", name=f"ps_{n0}_{b}")
                    for k in range(ko_bf):
                        for m in range(M2):
                            nc.tensor.matmul(
                                ps[:, m * NB : (m + 1) * NB],
                                lhsT=xchs[k][:, b * P : (b + 1) * P],
                                rhs=wt_sb[:, k, m * NB : (m + 1) * NB],
                                start=(k == 0),
                                stop=(not fp8_pairs and k == ko_bf - 1),
                            )
                    # fp8e4 DoubleRow tail: each instruction covers 2
                    # k-chunks ([p, 2, f] operands) at one bf16 matmul's cost.
                    for m in range(M2):
                        for q in range(fp8_pairs):
                            nc.tensor.matmul(
                                ps[:, m * NB : (m + 1) * NB],
                                lhsT=x8s[q][:, :, b * P : (b + 1) * P],
                                rhs=w8_sb[:, q, :, m * NB : (m + 1) * NB],
                                start=False,
                                stop=(q == fp8_pairs - 1),
                                perf_mode=mybir.MatmulPerfMode.DoubleRow,
                            )
                    # Bias-add then Relu, both on DVE. Relu as
                    # tensor_scalar_max(0.0) with an immediate: no Scalar
                    # activation -> no ACT_TABLE_LOAD / const-AP memsets in
                    # the NEFF init chain, which otherwise gate the first
                    # data DMAs ~6us at kernel start.
                    osb = opool.tile([P, E], f32, tag="osb")
                    nc.vector.tensor_add(out=osb, in0=ps, in1=bias_sb)
                    nc.vector.tensor_scalar_max(osb, osb, 0.0)
                    nc.sync.dma_start(
                        out[n0 + b * P : n0 + (b + 1) * P, :], osb
                    )

    nc.compile()
    return nc


def _get_nc(dt_name, fp8_pairs):
    key = (dt_name, fp8_pairs)
    if key not in _NC_CACHE:
        _NC_CACHE[key] = _build_nc(dt_name, fp8_pairs)
    return _NC_CACHE[key]


def _compose_weights(Wa, ba, Wv, bv, Wi, bi, Wo, bo, Wf, bf):
    f6 = lambda x: np.asarray(x, dtype=np.float64)
    Wvo = f6(Wo) @ f6(Wi[2 * E :])
    bvo = f6(Wo) @ f6(bi[2 * E :]) + f6(bo)
    Wf1, Wf2 = f6(Wf[:, :E]), f6(Wf[:, E:])
    Wfv = Wf1 @ Wvo  # applied to visual_e for audio_att
    Wfa = Wf2 @ Wvo  # applied to audio_e for visual_att
    Waa = Wfa @ f6(Wa)  # [E, 2048] applied to audio
    Wva = Wfv @ f6(Wv)  # [E, 2048] applied to visual
    b = Wfa @ f6(ba) + Wfv @ f6(bv) + (Wf1 + Wf2) @ bvo + f6(bf)
    wt = np.ascontiguousarray(
        np.concatenate([Waa, Wva], axis=1).T, dtype=np.float32
    )  # [K, E]
    return wt, b.astype(np.float32)


def kernel(audio, visual, Wa, ba, Wv, bv, Wi, bi, Wo, bo, Wf, bf):
    global LAST_RESULTS
    wt, bias = _compose_weights(Wa, ba, Wv, bv, Wi, bi, Wo, bo, Wf, bf)
    bias_bc = np.ascontiguousarray(np.broadcast_to(bias, (P, E)), np.float32)
    np_mm = mybir.dt.np(
        {
            "f32": mybir.dt.float32,
            "f32r": mybir.dt.float32r,
            "bf16": mybir.dt.bfloat16,
        }[DT_NAME]
    )
    np_fp8 = mybir.dt.np(mybir.dt.float8e4)
    audio = np.asarray(audio, dtype=np.float32)
    visual = np.asarray(visual, dtype=np.float32)

    wt_bf = wt[:KB8].astype(np_mm)
    in_maps = []
    if FP8_PAIRS:
        # paired fp8 weight chunks, [p, q, s, e] <- row KB8 + q*256 + s*128 + p
        w8 = np.ascontiguousarray(
            (wt[KB8:] * FP8_SCALE)
            .reshape(FP8_PAIRS, 2, P, E)
            .transpose(2, 0, 1, 3)
            .astype(np_fp8)
        )
    for c in range(N_CORES):
        rows = slice(c * BC, (c + 1) * BC)
        x_c = np.concatenate([audio[rows].T, visual[rows].T], axis=0)
        xt_c = np.ascontiguousarray(x_c[:KB8]).astype(np_mm)
        im = {"xt": xt_c, "wt": wt_bf, "bias": bias_bc}
        if FP8_PAIRS:
            im["x8t"] = np.ascontiguousarray(
                (x_c[KB8:] * (1.0 / FP8_SCALE))
                .reshape(FP8_PAIRS, 2, P, BC)
                .transpose(0, 2, 1, 3)
                .astype(np_fp8)
            )
            im["w8t"] = w8
        in_maps.append(im)

    nc = _get_nc(DT_NAME, FP8_PAIRS)
    trace = os.environ.get("KMM_TRACE", "0") == "1"
    kwargs = {}
    if os.environ.get("KMM_TRACE_ALL", "0") == "1":
        kwargs["trace_cores"] = list(range(N_CORES))
    res = run_bass_kernel_spmd(
        nc, in_maps, core_ids=list(range(N_CORES)), trace=trace, **kwargs
    )
    LAST_RESULTS = res
    out = np.concatenate([r["out"] for r in res.results], axis=0)
    return np.ascontiguousarray(out, dtype=np.float32)
